# revision 11
# baseline (speedup 1.0000x reference)
"""BREWA (bit-witness) attention on 8 TRN2 NeuronCores.

Sharding: core c = (batch b, head-group g) with b = c // 2, g = c % 2.
Each core computes its batch's attention for 8 of the 16 heads plus the
partial output projection over those heads' Wo columns; the host sums the
two partial projections per batch (the "all-reduce" is 2-way, done on CPU).

v2 vs v1:
  * Encoder fusion: Q/K are only consumed through their 32-bit witness
    encodings, so the host precomputes A_q[h] = Wq_h^T @ W_enc_h (1024x32)
    and the kernel computes q_encT = tanh(A_q^T @ x^T) in ONE PE pass --
    the Q/K projections never materialize.  Same for K.
  * exp split across 3 engines: ACT runs real Exp; DVE and GPSIMD run a
    Schraudolph fast-exp (int16 bitcast trick: i16 = trunc(s*A + B) viewed
    as bf16 ~= exp(s*SCALE), ripple ~2%; softmax cancels the bias).
    Round-robin per kt so consecutive score tiles drain concurrently.
  * Output projection DMAs straight from PSUM to DRAM (no DVE copy).

Per-core dataflow (all matmuls bf16, fp32 PSUM accumulate):
  q_encT,k_encT [128, 2048] tiles: 4 heads x 32 bits on partitions, seq free
  V [2048, 520]: seq on partitions, 65 cols/head (64 V dims + ones row)
  per (head pair, q-tile 512, k-tile 128):
    ST[k,q] via 2 row-strip K=32 matmuls -> st psum [128, 1024]
    exp(ST/sqrt32) -> SBUF bf16/int16 (engine by kt round-robin)
    att[r] += V_aug[kt,h].T @ expST   (psum [65,512]; row 64 = Z)
  normalize: DVE reciprocal(Z) -> GPSIMD partition_broadcast -> DVE mul
  y = c_T.T @ WoT_g  (per-core partial, fp32, DMA from PSUM)
"""

import numpy as np
import ml_dtypes

import concourse.bacc as bacc
import concourse.bass as bass
import concourse.mybir as mybir
import concourse.tile as tile
from concourse.bass_utils import run_bass_kernel_spmd

B, N, D = 4, 2048, 1024
H, HD, MB = 16, 64, 32
NCORES = 8
HPG = 8              # heads per group (per core)
GD = HPG * HD        # 512 head dims per group
NE = HPG * MB        # 256 encoder dims per group
SCALE = float(1.0 / np.sqrt(MB))
# Schraudolph fast-exp constants (int16/bf16): i = trunc(s*EXP_A + EXP_B),
# bitcast bf16 ~= exp(s*SCALE).  Bias sigma cancels in softmax; 0.5 centers
# the truncation.
EXP_A = float(SCALE * 128.0 * np.log2(np.e))
EXP_B = float(128.0 * 127.0 - 0.5 * 128.0 * 0.0433)

bf16 = mybir.dt.bfloat16
f32 = mybir.dt.float32
i16 = mybir.dt.int16
BF = ml_dtypes.bfloat16
AF = mybir.ActivationFunctionType
ALU = mybir.AluOpType

KT_X = D // 128      # 8 contraction tiles over d_model
NT = N // 512        # 4 column tiles of 512 over sequence
NT128 = N // 128     # 16 row tiles of 128 over sequence
KT_C = GD // 128     # 4 contraction tiles over group head dims

# exp engine per kt: ACT 10, DVE 6 (GPSIMD cannot read PSUM, so Pool only
# does the normalize broadcasts).  phi = 6/16 Schraudolph share.
EXP_ENGINE = ["A", "D", "A", "D", "A", "D", "A", "D",
              "A", "D", "A", "D", "A", "A", "A", "A"]

TRACE = False        # set by test.py for profiling runs
TRACE_KW = {}
LAST_RESULTS = None
PHASE_LIMIT = "full"  # "qkv" | "attn" | "full" -- for sim phase ablation


def build(reps=1):
    nc = bacc.Bacc("TRN2", target_bir_lowering=False, debug=False,
                   num_devices=NCORES)
    xt = nc.dram_tensor("xt", [D, N], bf16, kind="ExternalInput").ap()
    aq = nc.dram_tensor("aq", [D, NE], bf16, kind="ExternalInput").ap()
    ak = nc.dram_tensor("ak", [D, NE], bf16, kind="ExternalInput").ap()
    wv = nc.dram_tensor("wv", [D, GD], bf16, kind="ExternalInput").ap()
    wo = nc.dram_tensor("wo", [GD, D], bf16, kind="ExternalInput").ap()
    y = nc.dram_tensor("y", [N, D], f32, kind="ExternalOutput").ap()

    with tile.TileContext(nc) as tc:
        with (
            tc.tile_pool(name="xtp", bufs=KT_X) as xt_pool,
            tc.tile_pool(name="ap", bufs=2 * KT_X) as a_pool,
            tc.tile_pool(name="wvp", bufs=KT_X) as wv_pool,
            tc.tile_pool(name="wop", bufs=KT_C) as wo_pool,
            tc.tile_pool(name="encp", bufs=4) as enc_pool,
            tc.tile_pool(name="vp", bufs=NT128) as v_pool,
            tc.tile_pool(name="expp", bufs=6) as exp_pool,
            tc.tile_pool(name="ctp", bufs=KT_C) as ct_pool,
            tc.tile_pool(name="smallp", bufs=8) as small_pool,
            tc.tile_pool(name="yp", bufs=3) as y_pool,
            tc.tile_pool(name="stp", bufs=2, space="PSUM") as st_pool,
            tc.tile_pool(name="bankp", bufs=4, space="PSUM") as bank_pool,
        ):
          for _rep in range(reps):
            # ---- input loads -------------------------------------------------
            def load_tiles(ap_dram, pool, cols, tag):
                tiles = []
                for k in range(KT_X):
                    t = pool.tile([128, cols], bf16, tag=tag)
                    nc.sync.dma_start(t[:], ap_dram[128 * k:128 * (k + 1), :])
                    tiles.append(t)
                return tiles

            ak_sb = load_tiles(ak, a_pool, NE, "a")
            aq_sb = load_tiles(aq, a_pool, NE, "a")
            # xt in column chunks so the first encoder group only waits on
            # the first 512 columns of each contraction tile
            xt_sb = []
            for k in range(KT_X):
                t = xt_pool.tile([128, N], bf16, tag="xt")
                xt_sb.append(t)
            for nt in range(NT):
                for k in range(KT_X):
                    nc.sync.dma_start(
                        xt_sb[k][:, 512 * nt:512 * (nt + 1)],
                        xt[128 * k:128 * (k + 1), 512 * nt:512 * (nt + 1)])
            wv_sb = load_tiles(wv, wv_pool, GD, "wv")
            wo_sb = []
            for k in range(KT_C):
                t = wo_pool.tile([128, D], bf16, tag="wo")
                nc.sync.dma_start(t[:], wo[128 * k:128 * (k + 1), :])
                wo_sb.append(t)

            # ---- encodings: [128, 2048] per quad/side, fused A = W^T W_enc --
            q_enc = [enc_pool.tile([128, N], bf16, tag="enc", name=f"qenc{i}")
                     for i in range(2)]
            k_enc = [enc_pool.tile([128, N], bf16, tag="enc", name=f"kenc{i}")
                     for i in range(2)]

            def enc_nt(a_sb, et, mt, nt, name):
                ps = bank_pool.tile([128, 512], f32, tag="bank",
                                    name=f"eps_{name}_{nt}")
                for k in range(KT_X):
                    nc.tensor.matmul(
                        ps[:],
                        a_sb[k][:, 128 * mt:128 * (mt + 1)],
                        xt_sb[k][:, 512 * nt:512 * (nt + 1)],
                        start=(k == 0), stop=(k == KT_X - 1),
                        skip_group_check=True,
                    )
                nc.scalar.activation(et[:, 512 * nt:512 * (nt + 1)],
                                     ps[:], AF.Tanh)

            v_sb = [None] * NT128

            def ensure_v(nt):
                if v_sb[nt] is not None:
                    return v_sb[nt]
                t = v_pool.tile([128, HPG * 65], bf16, tag="v", name=f"v{nt}")
                ps = bank_pool.tile([128, 512], f32, tag="bank",
                                    name=f"ps_v{nt}")
                for k in range(KT_X):
                    nc.tensor.matmul(
                        ps[:],
                        xt_sb[k][:, 128 * nt:128 * (nt + 1)],
                        wv_sb[k][:],
                        start=(k == 0), stop=(k == KT_X - 1),
                        skip_group_check=True,
                    )
                vv = t[:, :].rearrange("p (h s) -> p h s", h=HPG)
                nc.vector.tensor_copy(
                    vv[:, :, 0:64],
                    ps[:, :].rearrange("p (h s) -> p h s", h=HPG),
                )
                nc.vector.memset(vv[:, :, 64:65], 1.0)
                v_sb[nt] = t
                return t

            # ---- c_T accumulator tiles: [512 head dims, 2048 seq] -----------
            ct_sb = [ct_pool.tile([128, N], bf16, tag="ct", name=f"ct{i}")
                     for i in range(KT_C)]

            # ---- attention: flat lag-2 software pipeline --------------------
            # PE's queue is in-order, so attV(step) is emitted LAG steps
            # behind ST(step): the exp engines get 2 ST windows to drain a
            # score tile before PE needs it, keeping PE gapless.
            LAG = 2
            att_ps = {}      # (p, qt) -> [att_r0, att_r1] psum tiles
            exp_tiles = {}   # step index -> (rhs AP provider, engine)

            def emit_st_exp(i, p, qt, kt):
                st = st_pool.tile([128, N // 2], f32, tag="st")
                qd = p // 2
                for r in range(2):
                    a = 2 * (p % 2) + r
                    nc.tensor.matmul(
                        st[:, 512 * r:512 * (r + 1)],
                        k_enc[qd][32 * a:32 * (a + 1),
                                  128 * kt:128 * (kt + 1)],
                        q_enc[qd][32 * a:32 * (a + 1),
                                  512 * qt:512 * (qt + 1)],
                        start=True, stop=True,
                        tile_position=(32 * a, 0),
                        skip_group_check=True,
                    )
                eng = EXP_ENGINE[kt]
                if eng == "A":
                    ex = exp_pool.tile([128, N // 2], bf16, tag="exp")
                    nc.scalar.activation(ex[:], st[:], AF.Exp, scale=SCALE)
                else:
                    ex = exp_pool.tile([128, N // 2], i16, tag="exp")
                    nc.vector.tensor_scalar(ex[:], st[:], EXP_A, EXP_B,
                                            ALU.mult, ALU.add)
                exp_tiles[i] = (ex, eng)
                if p == 0:
                    ensure_v(min(kt + 2, NT128 - 1))

            def normalize(p, qt):
                att = att_ps.pop((p, qt))
                for r in range(2):
                    h = 2 * p + r
                    recip = small_pool.tile([1, 512], f32, tag="recip")
                    nc.vector.reciprocal(recip[:], att[r][64:65, :])
                    bc = small_pool.tile([64, 512], f32, tag="bc")
                    nc.gpsimd.partition_broadcast(bc[:], recip[:])
                    u = 64 * (h % 2)
                    if u == 0:
                        nc.vector.tensor_mul(
                            ct_sb[h // 2][0:64, 512 * qt:512 * (qt + 1)],
                            att[r][0:64, :], bc[:])
                    else:
                        tmp = small_pool.tile([64, 512], bf16, tag="tmp")
                        nc.vector.tensor_mul(tmp[:], att[r][0:64, :], bc[:])
                        nc.sync.dma_start(
                            ct_sb[h // 2][64:128, 512 * qt:512 * (qt + 1)],
                            tmp[:])

            def emit_attv(i, p, qt, kt):
                ex, eng = exp_tiles.pop(i)
                if kt == 0:
                    att_ps[(p, qt)] = [
                        bank_pool.tile([65, 512], f32, tag="bank",
                                       name=f"att{p}_{qt}_{r}")
                        for r in range(2)]
                att = att_ps[(p, qt)]
                for r in range(2):
                    h = 2 * p + r
                    m = ex[:, 512 * r:512 * (r + 1)]
                    if eng != "A":
                        m = m.bitcast(bf16)
                    nc.tensor.matmul(
                        att[r][:],
                        ensure_v(kt)[:, 65 * h:65 * h + 65],
                        m,
                        start=(kt == 0), stop=(kt == NT128 - 1),
                        skip_group_check=True,
                    )
                if kt == NT128 - 1:
                    normalize(p, qt)
                    # fillers, emitted once block (p, qt) fully drains:
                    if p == 0:
                        enc_nt(ak_sb, k_enc[1], 1, qt, "kenc1")
                        enc_nt(aq_sb, q_enc[1], 1, qt, "qenc1")
                    elif p == 3 and qt > 0:
                        out_proj_qt(qt - 1)

            def attention_all():
                steps = [(p, qt, kt)
                         for p in range(4) for qt in range(NT)
                         for kt in range(NT128)]
                for i, (p, qt, kt) in enumerate(steps):
                    emit_st_exp(i, p, qt, kt)
                    if i >= LAG:
                        emit_attv(i - LAG, *steps[i - LAG])
                for i in range(len(steps) - LAG, len(steps)):
                    emit_attv(i, *steps[i])
                out_proj_qt(NT - 1)

            def out_proj_qt(qt):
                # y rows 512*qt .. 512*(qt+1): 4 m-tiles x 2 out-dim halves
                for mt in range(4 * qt, 4 * qt + 4):
                    for nt2 in range(2):
                        ps = bank_pool.tile([128, 512], f32, tag="bank",
                                            name=f"ps_y{mt}_{nt2}")
                        for k in range(KT_C):
                            nc.tensor.matmul(
                                ps[:],
                                ct_sb[k][:, 128 * mt:128 * (mt + 1)],
                                wo_sb[k][:, 512 * nt2:512 * (nt2 + 1)],
                                start=(k == 0), stop=(k == KT_C - 1),
                                skip_group_check=True,
                            )
                        yt = y_pool.tile([128, 512], f32, tag="y")
                        nc.scalar.copy(yt[:], ps[:])
                        nc.sync.dma_start(
                            y[128 * mt:128 * (mt + 1),
                              512 * nt2:512 * (nt2 + 1)],
                            yt[:])

            # ---- emission order (drives scheduler priority) -----------------
            # Ramp: k_enc quad 0 first (pair 0 needs the full k_enc row but
            # only q_enc's first 512 columns), then V 0/1.  Quad-1 encodings
            # are emitted inside pair 0's windows; output projection chases
            # pair 3 per q-tile.
            for nt in range(NT):
                enc_nt(ak_sb, k_enc[0], 0, nt, "kenc0")
                enc_nt(aq_sb, q_enc[0], 0, nt, "qenc0")
            for nt in range(2):
                ensure_v(nt)
            if PHASE_LIMIT == "qkv":
                continue
            attention_all()
    nc.finalize()
    return nc


_nc_cache = None


def make_in_maps(inputs):
    x = np.asarray(inputs["x"], dtype=np.float32)
    Wq = np.asarray(inputs["Wq"], dtype=np.float32)
    Wk = np.asarray(inputs["Wk"], dtype=np.float32)
    Wv = np.asarray(inputs["Wv"], dtype=np.float32)
    We = np.asarray(inputs["W_enc"], dtype=np.float32)
    Wo = np.asarray(inputs["Wo"], dtype=np.float32)

    # Fused encoder weights: A_side[h] = W_h^T @ We_h  [1024, 32]
    Aq = np.concatenate(
        [Wq[64 * h:64 * (h + 1), :].T @ We[h] for h in range(H)], axis=1)
    Ak = np.concatenate(
        [Wk[64 * h:64 * (h + 1), :].T @ We[h] for h in range(H)], axis=1)

    xts = [np.ascontiguousarray(x[b].T).astype(BF) for b in range(B)]
    in_maps = []
    for c in range(NCORES):
        b, g = divmod(c, 2)
        gs = g * GD
        es = g * NE
        in_maps.append({
            "xt": xts[b],
            "aq": np.ascontiguousarray(Aq[:, es:es + NE]).astype(BF),
            "ak": np.ascontiguousarray(Ak[:, es:es + NE]).astype(BF),
            "wv": np.ascontiguousarray(Wv[gs:gs + GD, :].T).astype(BF),
            "wo": np.ascontiguousarray(Wo[:, gs:gs + GD].T).astype(BF),
        })
    return in_maps


def kernel(**inputs):
    global _nc_cache, LAST_RESULTS
    if _nc_cache is None:
        _nc_cache = build()
    nc = _nc_cache
    in_maps = make_in_maps(inputs)

    res = run_bass_kernel_spmd(
        nc, in_maps, core_ids=list(range(NCORES)),
        trace=TRACE, **TRACE_KW)
    LAST_RESULTS = res

    out = np.empty((B, N, D), dtype=np.float32)
    for b in range(B):
        out[b] = res.results[2 * b]["y"] + res.results[2 * b + 1]["y"]
    return out
